# revision 2
# baseline (speedup 1.0000x reference)
"""Trainium2 Bass kernel for nn_CrossAttention (B=2, N=2048, D=768, H=12).

Sharding: (batch, head-group) across 8 cores — core c handles batch c//4 and
heads [3g, 3g+2] where g = c%4. Attention is fully local per (batch, head).

v2 design (all matmuls bf16; inputs/weights converted to bf16 on HOST):
  - x1[b].T / x2[b].T and the weight slices are DMA'd directly as bf16 (no
    on-chip rounding copies).
  - qT/kT [pd, N] per head via PE, bias added during PSUM->SBUF (DVE).
    Heads 0/1 live on partitions 0:64 / 64:128 of one [128, N] tile.
  - Attention runs i-half-outer (ih in {0,1} covering 1024 queries), heads
    0 and 1 interleaved per key-chunk jc so their K=64 S^T matmuls land on
    different PE row-halves (tile_position (0,0) vs (64,0)) and execute
    concurrently. exp on ACT (the ~100us bottleneck engine) -> bf16 SBUF.
    AV accumulates [65, 512] per (head, i-quarter) in PSUM; row 64 is the
    softmax denominator via a ones-column in v'.
  - v projection (natural [j, pd] layout + ones col) is interleaved into the
    ih=0 jc loop; q2/k2 projections (M=64, col-tiled pair) into the ih=1 loop,
    so PE fills its exp-wait gaps and ACT starts as early as possible.
  - Head 2 runs after heads 0/1, untiled (that phase is ACT-bound anyway).
  - Division: DVE reciprocal of the denominator row + gpsimd partition
    broadcast + DVE multiply, then DMA out [pd, N] per head.
PSUM: tag "ps" 2 bufs x [128,1024] f32 (4 banks, S tiles + vproj) + tag "po"
4 bufs x [128,512] (4 banks, AV accumulators + projection PSUM) = 8 banks.
"""

import sys

if "/opt/trn_rl_repo" not in sys.path:
    sys.path.insert(0, "/opt/trn_rl_repo")

import numpy as np

import concourse.bass as bass
import concourse.tile as tile
from concourse import bacc, mybir
from concourse.bass_utils import run_bass_kernel_spmd

F32 = mybir.dt.float32
BF16 = mybir.dt.bfloat16
AF = mybir.ActivationFunctionType

B, N, D, H, PD = 2, 2048, 768, 12, 64
HPC = 3  # heads per core
KC = 6  # contraction chunks: 768 / 128
NJ = 16  # j (key) chunks of 128
WV = HPC * PD  # v-projection rhs width (192)
VW = HPC * (PD + 1)  # v' block width per j-tile (195)
WQK = HPC * PD  # 192

# test harness hooks
TRACE = False
LAST_RESULTS = None

_cache: dict = {}


def _emit(tc, xq_t, xkv_t, wq_t, wk_t, wv_t, bq, bk, bv, o_t, loop_iters=1):
    if loop_iters > 1:
        with tc.For_i(0, loop_iters, 1):
            _emit_body(tc, xq_t, xkv_t, wq_t, wk_t, wv_t, bq, bk, bv, o_t)
    else:
        _emit_body(tc, xq_t, xkv_t, wq_t, wk_t, wv_t, bq, bk, bv, o_t)


def _emit_body(tc, xq_t, xkv_t, wq_t, wk_t, wv_t, bq, bk, bv, o_t):
    nc = tc.nc

    import contextlib

    with contextlib.ExitStack() as ctx:
        persist = ctx.enter_context(tc.tile_pool(name="persist", bufs=1))
        expp = ctx.enter_context(tc.tile_pool(name="expp", bufs=3))
        outp = ctx.enter_context(tc.tile_pool(name="outp", bufs=2))
        smallp = ctx.enter_context(tc.tile_pool(name="smallp", bufs=2))
        ps_pool = ctx.enter_context(tc.tile_pool(name="ps", bufs=2, space="PSUM"))
        po_pool = ctx.enter_context(tc.tile_pool(name="po", bufs=4, space="PSUM"))

        # ---- inputs: direct bf16 DMA ----
        xkv_sb = persist.tile([128, KC * N], BF16)
        nc.sync.dma_start(
            xkv_sb[:].rearrange("p (kc n) -> p kc n", kc=KC),
            xkv_t.rearrange("(kc p) n -> p kc n", p=128),
        )
        xq_sb = persist.tile([128, KC * N], BF16)
        nc.sync.dma_start(
            xq_sb[:].rearrange("p (kc n) -> p kc n", kc=KC),
            xq_t.rearrange("(kc p) n -> p kc n", p=128),
        )

        def load_w(wdram, wcols):
            w_sb = persist.tile([128, KC * wcols], BF16, name=wdram.name + "_sb")
            nc.sync.dma_start(
                w_sb[:].rearrange("p (kc w) -> p kc w", kc=KC),
                wdram.rearrange("(kc p) w -> p kc w", p=128),
            )
            return w_sb

        wk_sb = load_w(wk_t, WQK)
        wq_sb = load_w(wq_t, WQK)
        wv_sb = load_w(wv_t, WV)

        # ---- biases ----
        bq_sb = persist.tile([128, 2], F32)
        bk_sb = persist.tile([128, 2], F32)
        nc.sync.dma_start(bq_sb[:, 0:1], bq[0:128, :])
        nc.sync.dma_start(bq_sb[0:64, 1:2], bq[128:192, :])
        nc.sync.dma_start(bk_sb[:, 0:1], bk[0:128, :])
        nc.sync.dma_start(bk_sb[0:64, 1:2], bk[128:192, :])
        bv_sb = persist.tile([1, WV], BF16)
        nc.sync.dma_start(bv_sb[:], bv[:])

        # ones row [1, 128] bf16 for the v-bias rank-1 matmul
        ones_row_f = persist.tile([1, 128], F32)
        nc.vector.memset(ones_row_f[:], 1.0)
        ones_row = persist.tile([1, 128], BF16)
        nc.vector.tensor_copy(ones_row[:], ones_row_f[:])

        # ones [128, 48] f32 source for v' ones-columns
        ones48 = persist.tile([128, 48], F32)
        nc.vector.memset(ones48[:], 1.0)

        # ---- q/k projections for heads 0,1 (M=128: two heads stacked) ----
        qT01 = persist.tile([128, N], BF16)
        kT01 = persist.tile([128, N], BF16)
        qT2 = persist.tile([64, N], BF16)
        kT2 = persist.tile([64, N], BF16)

        def proj01(w_sb, x_sb, b_sb, out_t, ic):
            ps = po_pool.tile([128, 512], F32, tag="po", name=f"pj_{ic}")
            for kc in range(KC):
                nc.tensor.matmul(
                    ps[:],
                    w_sb[:, kc * WQK : kc * WQK + 128],
                    x_sb[:, kc * N + ic * 512 : kc * N + (ic + 1) * 512],
                    start=(kc == 0),
                    stop=(kc == KC - 1),
                )
            nc.vector.tensor_scalar_add(
                out_t[:, ic * 512 : (ic + 1) * 512], ps[:], b_sb[:, 0:1]
            )

        for ic in range(4):
            proj01(wk_sb, xkv_sb, bk_sb, kT01, ic)
        for ic in range(4):
            proj01(wq_sb, xq_sb, bq_sb, qT01, ic)

        # q2/k2 projection for head 2 (M=64 each) — col-tiled pair sharing one
        # PSUM tile: q2 -> partitions 0:64 (col grp 0), k2 -> 64:128 (col grp 64)
        def proj2(ic):
            ps = po_pool.tile([128, 512], F32, tag="po", name=f"pj2_{ic}")
            for kc in range(KC):
                nc.tensor.matmul(
                    ps[0:64, :],
                    wq_sb[:, kc * WQK + 128 : kc * WQK + 192],
                    xq_sb[:, kc * N + ic * 512 : kc * N + (ic + 1) * 512],
                    start=(kc == 0),
                    stop=(kc == KC - 1),
                )
            for kc in range(KC):
                nc.tensor.matmul(
                    ps[64:128, :],
                    wk_sb[:, kc * WQK + 128 : kc * WQK + 192],
                    xkv_sb[:, kc * N + ic * 512 : kc * N + (ic + 1) * 512],
                    start=(kc == 0),
                    stop=(kc == KC - 1),
                )
            nc.vector.tensor_scalar_add(
                qT2[:, ic * 512 : (ic + 1) * 512], ps[0:64, :], bq_sb[0:64, 1:2]
            )
            nc.vector.tensor_scalar_add(
                kT2[:, ic * 512 : (ic + 1) * 512], ps[64:128, :], bk_sb[0:64, 1:2]
            )

        # ---- v' (natural layout, 3 heads of 64 + ones col per head) ----
        v_sb = persist.tile([128, NJ * VW], BF16)

        def vproj(jt):
            ps = ps_pool.tile([128, WV], F32, tag="ps", name=f"vp_{jt}")
            for kc in range(KC):
                nc.tensor.matmul(
                    ps[:],
                    xkv_sb[:, kc * N + jt * 128 : kc * N + (jt + 1) * 128],
                    wv_sb[:, kc * WV : (kc + 1) * WV],
                    start=(kc == 0),
                    stop=False,
                )
            nc.tensor.matmul(ps[:], ones_row[:], bv_sb[:], start=False, stop=True)
            src = ps[:].rearrange("p (h c) -> p h c", h=HPC)
            dstv = v_sb[:, jt * VW : (jt + 1) * VW].rearrange(
                "p (h c) -> p h c", h=HPC
            )[:, :, 0:PD]
            nc.vector.tensor_copy(dstv, src)

        def set_v_ones():
            dst_ones = v_sb[:].rearrange("p (g c) -> p g c", c=PD + 1)[
                :, :, PD : PD + 1
            ]
            nc.vector.tensor_copy(
                dst_ones, ones48[:].rearrange("p (g o) -> p g o", o=1)
            )

        def vp(jc, h):
            return v_sb[:, jc * VW + h * (PD + 1) : jc * VW + (h + 1) * (PD + 1)]

        def divide_out(po_t, h, ih, q):
            recip = smallp.tile([1, 512], F32, tag="rcp")
            nc.vector.reciprocal(recip[:], po_t[PD : PD + 1, :])
            bcast = smallp.tile([64, 512], F32, tag="bc")
            nc.gpsimd.partition_broadcast(bcast[:], recip[:])
            out_sb = outp.tile([64, 512], F32, tag="out")
            nc.vector.tensor_mul(out_sb[:], po_t[0:PD, :], bcast[:])
            c0 = ih * 1024 + q * 512
            nc.sync.dma_start(o_t[h, :, c0 : c0 + 512], out_sb[:])

        # ---- attention heads 0/1, i-half-outer, heads interleaved per jc ----
        for ih in range(2):
            po = [
                [
                    po_pool.tile([128, 512], F32, tag="po", name=f"po_h{hh}_{ih}_{q}")
                    for q in range(2)
                ]
                for hh in range(2)
            ]
            if ih == 0:
                vproj(0)
            for jc in range(NJ):
                pss = []
                for hh in range(2):
                    p0, p1 = hh * 64, hh * 64 + 64
                    ptile = ps_pool.tile(
                        [128, 1024], F32, tag="ps", name=f"ss_h{hh}_{ih}_{jc}"
                    )
                    for q in range(2):
                        ic = ih * 2 + q
                        nc.tensor.matmul(
                            ptile[:, q * 512 : (q + 1) * 512],
                            kT01[p0:p1, jc * 128 : (jc + 1) * 128],
                            qT01[p0:p1, ic * 512 : (ic + 1) * 512],
                            start=True,
                            stop=True,
                        )
                    pss.append(ptile)
                exs = []
                for hh in range(2):
                    ex = expp.tile([128, 1024], BF16, tag="ex")
                    nc.scalar.activation(ex[:], pss[hh][:], AF.Exp)
                    exs.append(ex)
                # PE filler while ACT computes exp
                if ih == 0:
                    if jc == 0:
                        set_v_ones()
                    if jc < NJ - 1:
                        vproj(jc + 1)
                else:
                    if jc < 4:
                        proj2(jc)
                for hh in range(2):
                    for q in range(2):
                        nc.tensor.matmul(
                            po[hh][q][0 : PD + 1, :],
                            vp(jc, hh),
                            exs[hh][:, q * 512 : (q + 1) * 512],
                            start=(jc == 0),
                            stop=(jc == NJ - 1),
                        )
            for hh in range(2):
                for q in range(2):
                    divide_out(po[hh][q], hh, ih, q)

        # ---- attention head 2 (untiled; phase is ACT-bound) ----
        for ih in range(2):
            po2 = [
                po_pool.tile([128, 512], F32, tag="po", name=f"po2_{ih}_{q}")
                for q in range(2)
            ]
            for jc in range(NJ):
                ptile = ps_pool.tile([128, 1024], F32, tag="ps", name=f"s2_{ih}_{jc}")
                for q in range(2):
                    ic = ih * 2 + q
                    nc.tensor.matmul(
                        ptile[:, q * 512 : (q + 1) * 512],
                        kT2[:, jc * 128 : (jc + 1) * 128],
                        qT2[:, ic * 512 : (ic + 1) * 512],
                        start=True,
                        stop=True,
                    )
                ex = expp.tile([128, 1024], BF16, tag="ex")
                nc.scalar.activation(ex[:], ptile[:], AF.Exp)
                for q in range(2):
                    nc.tensor.matmul(
                        po2[q][0 : PD + 1, :],
                        vp(jc, 2),
                        ex[:, q * 512 : (q + 1) * 512],
                        start=(jc == 0),
                        stop=(jc == NJ - 1),
                    )
            for q in range(2):
                divide_out(po2[q], 2, ih, q)


def _build(loop_iters=1):
    key = ("nc", loop_iters)
    if key in _cache:
        return _cache[key]
    nc = bacc.Bacc("TRN2", target_bir_lowering=False, debug=False, num_devices=8)
    xq_t = nc.dram_tensor("xq_t", [D, N], BF16, kind="ExternalInput").ap()
    xkv_t = nc.dram_tensor("xkv_t", [D, N], BF16, kind="ExternalInput").ap()
    wq_t = nc.dram_tensor("wq_t", [D, WQK], BF16, kind="ExternalInput").ap()
    wk_t = nc.dram_tensor("wk_t", [D, WQK], BF16, kind="ExternalInput").ap()
    wv_t = nc.dram_tensor("wv_t", [D, WV], BF16, kind="ExternalInput").ap()
    bq = nc.dram_tensor("bq", [WQK, 1], F32, kind="ExternalInput").ap()
    bk = nc.dram_tensor("bk", [WQK, 1], F32, kind="ExternalInput").ap()
    bv = nc.dram_tensor("bv", [1, WV], BF16, kind="ExternalInput").ap()
    o_t = nc.dram_tensor("o_t", [HPC, PD, N], F32, kind="ExternalOutput").ap()
    with tile.TileContext(nc) as tc:
        _emit(tc, xq_t, xkv_t, wq_t, wk_t, wv_t, bq, bk, bv, o_t, loop_iters)
    nc.compile()
    _cache[key] = nc
    return nc


def _shard(x1, x2, Wq, bq, Wkv, bkv):
    import ml_dtypes

    bf16 = ml_dtypes.bfloat16
    in_maps = []
    for c in range(8):
        b, g = divmod(c, 4)
        hd = slice(192 * g, 192 * (g + 1))
        in_maps.append(
            {
                "xq_t": np.ascontiguousarray(x2[b].T).astype(bf16),
                "xkv_t": np.ascontiguousarray(x1[b].T).astype(bf16),
                "wq_t": np.ascontiguousarray(Wq[hd].T).astype(bf16),
                "wk_t": np.ascontiguousarray(Wkv[hd].T).astype(bf16),
                "wv_t": np.ascontiguousarray(
                    Wkv[D + hd.start : D + hd.stop].T
                ).astype(bf16),
                "bq": np.ascontiguousarray(bq[hd].reshape(-1, 1)),
                "bk": np.ascontiguousarray(bkv[hd].reshape(-1, 1)),
                "bv": np.ascontiguousarray(
                    bkv[D + hd.start : D + hd.stop].reshape(1, -1)
                ).astype(bf16),
            }
        )
    return in_maps


def kernel(x1, x2, Wq, bq, Wkv, bkv):
    global LAST_RESULTS
    x1 = np.asarray(x1, dtype=np.float32)
    x2 = np.asarray(x2, dtype=np.float32)
    Wq = np.asarray(Wq, dtype=np.float32)
    bq = np.asarray(bq, dtype=np.float32)
    Wkv = np.asarray(Wkv, dtype=np.float32)
    bkv = np.asarray(bkv, dtype=np.float32)

    nc = _build()
    in_maps = _shard(x1, x2, Wq, bq, Wkv, bkv)
    res = run_bass_kernel_spmd(nc, in_maps, core_ids=list(range(8)), trace=TRACE)
    LAST_RESULTS = res

    out = np.empty((B, H, N, PD), np.float32)
    for c in range(8):
        b, g = divmod(c, 4)
        ot = res.results[c]["o_t"]  # (3, 64, 2048)
        out[b, 3 * g : 3 * g + 3] = ot.transpose(0, 2, 1)
    return out.reshape(B, N, D)
